# revision 31
# baseline (speedup 1.0000x reference)
"""Trainium2 Bass kernel for nn_Attention_38276748542551.

Llama-style GQA attention block (DIM=4096, 32 q-heads, 8 kv-heads, hd=128,
b=2, s=2048, start_pos=0), tensor-parallel over heads across 8 NeuronCores:
each core owns 4 q-heads / 1 kv-head (wq/wk/wv output-dim shard, wo
input-dim shard) and computes a full [b*s, 4096] partial of the wo output;
the all-reduce is done on the host after gathering the 8 partials.

vs the fp32r baseline: all matmuls in bf16 (same 1 cycle/row PE rate as
fp32r but half the DMA bytes and SBUF), Q^T kept resident in SBUF (no DRAM
spill), rope pair-swap moved off the PE onto partition-offset DVE multiplies,
softmax denominator via PE broadcast-sum matmuls (ones[128x128]^T @ exp-tile
accumulated in PSUM alongside PV — every output row holds the column sum, so
no per-head row-sum/copy/broadcast chain), causal diagonal tiles trimmed to
their live column range, PV/bsum matmuls software-pipelined 2 tiles behind
the score matmuls, and the wo projection of block g-1 paced one instruction
at a time into block g's score stream as PE filler work.

Device dataflow per core:
  phase 1 (per 512-token block): Q/K/V projections (weight k-tile stationary,
  x^T moving) -> feature-major [feat, tok] PSUM -> rope via two half-partition
  DVE multiplies against [-sin,+sin] tables (pair-deinterleave baked into the
  weight sharding) -> Q^T and K^T resident in SBUF (bf16), V PE-transposed to
  token-major SBUF. Block 0 interleaves all six accumulators per k-tile (DMA
  warm-up); later blocks run six sequential 32-matmul streams (V, K, Q0..Q3)
  so accumulator banks stop ~7us apart and each PSUM-copy + rope pipeline
  (and the deferred one-per-stream V-transposes) hide under the next stream
  with zero PE bank-wait stalls -- including across the phase boundary.
  phase 2 (per batch, 512-token query block, head): scores transposed [sk,sq]
  per 128-sk-tile, diagonal tiles only computed on cols >= 128v -> causal
  mask add [128,128] on the diagonal wedge -> exp (ACT, bf16 out) -> PE
  accumulates V^T@exp into ps_o and ones^T@exp into ps_b (lagged 2 tiles,
  bsums drained before the last PVs so the reciprocal chain hides) ->
  approx-reciprocal + multiply -> attn^T -> wo matmuls + PSUM->SBUF bf16
  copies + DMA, interleaved into the next block's score stream.
"""
import sys
from collections import deque

import numpy as np

sys.path.insert(0, "/opt/trn_rl_repo")

import concourse.bass as bass  # noqa: E402
import concourse.tile as tile  # noqa: E402
from concourse import bacc, mybir  # noqa: E402
from concourse import bass_utils  # noqa: E402

import ml_dtypes  # noqa: E402

F32 = mybir.dt.float32
F32R = mybir.dt.float32r
BF16 = mybir.dt.bfloat16
AF = mybir.ActivationFunctionType
BF16NP = ml_dtypes.bfloat16

DIM = 4096
NK = DIM // 128          # contraction k-tiles (32)
NKQ = 4                  # k quarters
KPQ = NK // NKQ          # k-tiles per quarter (8)
HD = 128                 # head dim
NH_LOC = 4               # q heads per core
QDIM = NH_LOC * HD       # 512
KVDIM = 2 * HD           # K and V projected together, 256
N_CORES = 8
SOFTMAX_SCALE = 1.0 / np.sqrt(HD)


def build_nc(B=2, S=2048):
    """Build the per-core Bass program (identical across cores; data differs)."""
    NT = B * S // 128            # 128-token tiles total
    TPB = S // 128               # 128-token tiles per batch
    NQB = S // 512               # 512-token blocks per batch
    NTG = B * NQB                # 512-token blocks total

    nc = bacc.Bacc("TRN2", target_bir_lowering=False, debug=False,
                   enable_asserts=False, num_devices=N_CORES)

    x_t = nc.dram_tensor("x_t", [NTG, NKQ, KPQ, 128, 512], BF16, kind="ExternalInput").ap()
    wq_t = nc.dram_tensor("wq_t", [DIM, QDIM], BF16, kind="ExternalInput").ap()
    wkv_t = nc.dram_tensor("wkv_t", [DIM, KVDIM], BF16, kind="ExternalInput").ap()
    wo_t = nc.dram_tensor("wo_t", [QDIM, DIM], BF16, kind="ExternalInput").ap()
    cct_d = nc.dram_tensor("cct", [128, S], BF16, kind="ExternalInput").ap()
    sst_d = nc.dram_tensor("sst", [128, S], BF16, kind="ExternalInput").ap()
    ident_d = nc.dram_tensor("ident", [128, 128], BF16, kind="ExternalInput").ap()
    ones_d = nc.dram_tensor("ones", [128, 128], BF16, kind="ExternalInput").ap()
    maskt_d = nc.dram_tensor("maskt", [128, 128], F32, kind="ExternalInput").ap()
    out_d = nc.dram_tensor("out", [B * S, DIM], BF16, kind="ExternalOutput").ap()

    with tile.TileContext(nc) as tc:
        with tc.tile_pool(name="singles", bufs=1) as singles:
            ident = singles.tile([128, 128], BF16)
            ones_sb = singles.tile([128, 128], BF16)
            maskt = singles.tile([128, 128], F32)
            cct = singles.tile([128, S], BF16)
            sst = singles.tile([128, S], BF16)
            kt_sb = singles.tile([128, NT, 128], BF16)   # K^T: [hd, tile, tok]
            v_sb = singles.tile([128, NT, 128], BF16)    # V: [tok, tile, hd]
            qt_sb = singles.tile([128, NTG, NH_LOC, 512], BF16)  # Q^T resident
            wo_sb = singles.tile([128, NH_LOC, DIM], BF16)

            # ---------------- phase 1: projections + rope (feature-major) ----------------
            with tc.tile_pool(name="p1w", bufs=1) as p1w, \
                 tc.tile_pool(name="p1", bufs=6) as p1, \
                 tc.tile_pool(name="p1f", bufs=6) as p1f, \
                 tc.tile_pool(name="p1r", bufs=2) as p1r, \
                 tc.tile_pool(name="ps_acc", bufs=6, space="PSUM") as ps_accp, \
                 tc.tile_pool(name="ps_v", bufs=2, space="PSUM") as ps_vp:

                def load_xs(g, kq):
                    t_ = p1.tile([128, KPQ, 512], BF16, tag="xs")
                    nc.sync.dma_start(
                        out=t_, in_=x_t[g, kq].rearrange("k p t -> p k t"))
                    return t_

                # interleave first xs quarters with the weight chunk loads so
                # neither starves the other on the DMA queue; the very first
                # k-tile of weights and x goes in tiny loads so the PE can
                # start within ~2us of kernel start
                wq_sb = p1w.tile([128, NK, QDIM], BF16)
                wkv_sb = p1w.tile([128, NK, KVDIM], BF16)
                xs00 = p1.tile([128, KPQ, 512], BF16, tag="xs")

                def load_wq(k0, k1):
                    nc.sync.dma_start(
                        out=wq_sb[:, k0:k1, :],
                        in_=wq_t[k0 * 128:k1 * 128, :].rearrange("(k p) n -> p k n", p=128))

                def load_wkv(k0, k1):
                    nc.sync.dma_start(
                        out=wkv_sb[:, k0:k1, :],
                        in_=wkv_t[k0 * 128:k1 * 128, :].rearrange("(k p) n -> p k n", p=128))

                load_wq(0, 1)
                nc.sync.dma_start(
                    out=xs00[:, 0:1, :],
                    in_=x_t[0, 0, 0:1].rearrange("k p t -> p k t"))
                load_wkv(0, 1)
                load_wq(1, 4)
                nc.sync.dma_start(
                    out=xs00[:, 1:4, :],
                    in_=x_t[0, 0, 1:4].rearrange("k p t -> p k t"))
                load_wkv(1, 4)
                load_wq(4, 8)
                nc.sync.dma_start(
                    out=xs00[:, 4:8, :],
                    in_=x_t[0, 0, 4:8].rearrange("k p t -> p k t"))
                load_wkv(4, 8)
                xs_pre = [xs00]
                xs_pre.append(load_xs(0, 1))
                load_wq(8, 16)
                load_wkv(8, 16)
                xs_pre.append(load_xs(0, 2))
                load_wq(16, 24)
                load_wkv(16, 24)
                load_wq(24, 32)
                load_wkv(24, 32)
                xs_pre.append(load_xs(0, 3))
                nc.sync.dma_start(out=ident, in_=ident_d)
                nc.sync.dma_start(out=maskt, in_=maskt_d)
                nc.sync.dma_start(out=ones_sb, in_=ones_d)
                nc.sync.dma_start(out=cct, in_=cct_d)
                nc.sync.dma_start(out=sst, in_=sst_d)

                def load_wo(kk):
                    nc.sync.dma_start(
                        out=wo_sb[:, kk, :],
                        in_=wo_t[kk * 128:(kk + 1) * 128, :])

                def rope(g, j, acc_j):
                    # pair-swap realized as two half-partition DVE multiplies
                    # (the even/odd deinterleave is baked into the weight
                    # sharding). sst is half-rotated on the host so each
                    # half-multiply reads both SBUF inputs at the same base
                    # partition (verifier requirement); only the output shifts.
                    pos = (g % NQB) * 512
                    f = p1f.tile([128, 512], BF16, tag="f", name=f"f{g}_{j}")
                    nc.scalar.copy(f, acc_j)
                    t1 = p1r.tile([128, 512], BF16, tag="t1", name=f"t1_{g}_{j}")
                    nc.vector.tensor_mul(t1, f, cct[:, pos:pos + 512])
                    t2 = p1r.tile([128, 512], BF16, tag="t2", name=f"t2_{g}_{j}")
                    nc.vector.tensor_mul(t2[0:64, :], f[64:128, :], sst[64:128, pos:pos + 512])
                    nc.vector.tensor_mul(t2[64:128, :], f[0:64, :], sst[0:64, pos:pos + 512])
                    if j < NH_LOC:
                        nc.vector.tensor_add(qt_sb[:, g, j, :], t1, t2)
                    else:
                        nc.vector.tensor_add(
                            kt_sb[:, 4 * g:4 * g + 4, :].rearrange("p a t -> p (a t)"),
                            t1, t2)

                def vtail(g, acc_v):
                    # V: copy out of PSUM now; the four PE-transposes are
                    # deferred one-per-stream so they never head-block a
                    # projection stream
                    vf = p1f.tile([128, 512], BF16, tag="f", name=f"vf{g}")
                    nc.scalar.copy(vf, acc_v)

                    def vt(r, g=g, vf=vf):
                        ps_vt = ps_vp.tile([128, 1024], BF16, tag="vt",
                                           name=f"vt{g}_{r}")
                        nc.tensor.transpose(ps_vt[:, 0:128],
                                            vf[:, r * 128:(r + 1) * 128], ident)
                        nc.scalar.copy(v_sb[:, 4 * g + r, :], ps_vt[:, 0:128])
                    return [lambda r=r: vt(r) for r in range(4)]

                pending_vt = []
                for g in range(NTG):
                    acc = [ps_accp.tile([128, 512], F32, tag="acc", name=f"acc{g}_{j}") for j in range(6)]
                    if g == 0:
                        # startup block: interleave all six accumulators per
                        # k-tile so the weight/x DMA demand is spread out
                        for kq in range(NKQ):
                            xs = xs_pre[kq]
                            for k in range(KPQ):
                                kt = kq * KPQ + k
                                st = (kt == 0)
                                sp = (kt == NK - 1)
                                for h in range(NH_LOC):
                                    nc.tensor.matmul(acc[h], wq_sb[:, kt, h * 128:(h + 1) * 128],
                                                     xs[:, k, :], start=st, stop=sp)
                                nc.tensor.matmul(acc[4], wkv_sb[:, kt, 0:128],
                                                 xs[:, k, :], start=st, stop=sp)
                                nc.tensor.matmul(acc[5], wkv_sb[:, kt, 128:256],
                                                 xs[:, k, :], start=st, stop=sp)
                            if kq == 0:
                                xs_pre.append(load_xs(1, 0))
                                xs_pre.append(load_xs(1, 1))
                            elif kq == 1:
                                xs_pre.append(load_xs(1, 2))
                            elif kq == 2:
                                xs_pre.append(load_xs(1, 3))
                        pending_vt = vtail(0, acc[5])
                        for j in (4, 0, 1, 2, 3):
                            rope(0, j, acc[j])
                        xs_pre = xs_pre[NKQ:]
                    else:
                        # steady state: six sequential 32-matmul streams
                        # (V, K, Q0..Q3) so accumulator banks stop ~7us apart
                        # and each copy+rope pipeline hides under the next
                        # stream with the PE never waiting on a bank
                        xs_q = xs_pre[:NKQ]
                        for si, j in enumerate((5, 4, 0, 1, 2, 3)):
                            if j >= NH_LOC:
                                w_sb, col = wkv_sb, (j - NH_LOC) * 128
                            else:
                                w_sb, col = wq_sb, j * 128
                            for kt in range(NK):
                                kq, k = divmod(kt, KPQ)
                                nc.tensor.matmul(acc[j], w_sb[:, kt, col:col + 128],
                                                 xs_q[kq][:, k, :],
                                                 start=(kt == 0), stop=(kt == NK - 1))
                            if si == 0:
                                new_vt = vtail(g, acc[5])
                                if g + 1 < NTG:
                                    xs_pre.append(load_xs(g + 1, 0))
                                    xs_pre.append(load_xs(g + 1, 1))
                                if g - 2 in (0, 1, 2, 3):
                                    load_wo(g - 2)
                            else:
                                rope(g, j, acc[j])
                            if si == 1:
                                pending_vt.extend(new_vt)
                            if pending_vt:
                                pending_vt.pop(0)()
                            if si == 4 and g + 1 < NTG:
                                xs_pre.append(load_xs(g + 1, 2))
                            elif si == 5 and g + 1 < NTG:
                                xs_pre.append(load_xs(g + 1, 3))
                        xs_pre = xs_pre[NKQ:]
                for fvt in pending_vt:
                    fvt()

            # ------------- phase 2/3: attention (transposed scores) + wo -------------
            with tc.tile_pool(name="p2e", bufs=6) as p2e, \
                 tc.tile_pool(name="p2a", bufs=2) as p2a, \
                 tc.tile_pool(name="p2t", bufs=2) as p2t, \
                 tc.tile_pool(name="p2o", bufs=3) as p2o, \
                 tc.tile_pool(name="ps_s", bufs=3, space="PSUM") as ps_sp, \
                 tc.tile_pool(name="ps_o", bufs=2, space="PSUM") as ps_op, \
                 tc.tile_pool(name="ps_b", bufs=1, space="PSUM") as ps_bp, \
                 tc.tile_pool(name="ps_w", bufs=2, space="PSUM") as ps_wp:

                wo_ops = deque()

                def pump(n):
                    for _ in range(min(n, len(wo_ops))):
                        wo_ops.popleft()()

                def make_wo(b, qb, attn_t):
                    # one closure per instruction so the wo work can be paced
                    # into the next block's score stream as PE filler
                    ops = []
                    for r in range(4):
                        tt = b * TPB + qb * 4 + r
                        o_sb = p2o.tile([128, DIM], BF16, tag="o")
                        for n in range(DIM // 512):
                            cell = {}
                            for kk in range(NH_LOC):
                                def mm(kk=kk, n=n, r=r, tt=tt, attn_t=attn_t, cell=cell):
                                    if kk == 0:
                                        cell['ps'] = ps_wp.tile([128, 512], F32, tag="ps_w",
                                                                name=f"psw_{tt}_{n}")
                                    nc.tensor.matmul(cell['ps'], attn_t[:, kk, r, :],
                                                     wo_sb[:, kk, n * 512:(n + 1) * 512],
                                                     start=(kk == 0), stop=(kk == NH_LOC - 1))
                                ops.append(mm)

                            def cp(n=n, o_sb=o_sb, cell=cell):
                                nc.vector.tensor_copy(o_sb[:, n * 512:(n + 1) * 512], cell['ps'])
                            ops.append(cp)

                            if n % 2 == 1:
                                def dma(tt=tt, o_sb=o_sb, n=n):
                                    nc.sync.dma_start(
                                        out=out_d[tt * 128:(tt + 1) * 128,
                                                  (n - 1) * 512:(n + 1) * 512],
                                        in_=o_sb[:, (n - 1) * 512:(n + 1) * 512])
                                ops.append(dma)
                    return ops

                for b in range(B):
                    for qb in range(NQB):
                        g = b * NQB + qb
                        nt = 4 * (qb + 1)            # sk tiles for this block
                        attn_t = p2t.tile([128, NH_LOC, 4, 128], BF16, tag="attn_t")
                        for h in range(NH_LOC):
                            ps_o = ps_op.tile([128, 512], F32, tag="ps_o")
                            # denominator accumulator: ones^T @ et broadcast-sums
                            # exp over sk into every partition row
                            ps_b = ps_bp.tile([128, 512], F32, tag="ps_b")
                            ets = []

                            def emit_pv(t, ps_o=ps_o, ets=ets, b=b, qb=qb, nt=nt):
                                et, c0 = ets[t]
                                nc.tensor.matmul(ps_o[:, c0:], v_sb[:, b * TPB + t, :],
                                                 et[:, c0:],
                                                 start=(t == 0), stop=(t == nt - 1))

                            def emit_bs(t, ps_b=ps_b, ets=ets, nt=nt):
                                et, c0 = ets[t]
                                nc.tensor.matmul(ps_b[:, c0:], ones_sb,
                                                 et[:, c0:],
                                                 start=(t == 0), stop=(t == nt - 1))

                            for t in range(nt):
                                v = t - 4 * qb
                                c0 = 128 * v if v > 0 else 0
                                ps_s = ps_sp.tile([128, 512], F32, tag="ps_s")
                                nc.tensor.matmul(ps_s[:, c0:], kt_sb[:, b * TPB + t, :],
                                                 qt_sb[:, g, h, c0:],
                                                 start=True, stop=True)
                                if v >= 0:   # diagonal wedge: causal mask
                                    nc.vector.tensor_add(ps_s[:, c0:c0 + 128],
                                                         ps_s[:, c0:c0 + 128], maskt)
                                et = p2e.tile([128, 512], BF16, tag="et")
                                nc.scalar.activation(et[:, c0:], ps_s[:, c0:], AF.Exp,
                                                     scale=SOFTMAX_SCALE)
                                ets.append((et, c0))
                                if t >= 2:
                                    emit_bs(t - 2)
                                    emit_pv(t - 2)
                                pump(2)
                            # drain bsums first so the reciprocal chain hides
                            # under the last PV matmuls + filler
                            emit_bs(nt - 2)
                            emit_bs(nt - 1)
                            emit_pv(nt - 2)
                            emit_pv(nt - 1)
                            pump(1)
                            rb = p2a.tile([128, 512], F32, tag="rb")
                            nc.vector.reciprocal_approx_fast(out=rb, in_=ps_b)
                            nc.vector.tensor_mul(
                                attn_t[:, h].rearrange("p r t -> p (r t)"), ps_o, rb)
                            pump(2)
                        wo_ops.extend(make_wo(b, qb, attn_t))
                while wo_ops:
                    wo_ops.popleft()()

    nc.compile()
    return nc


def host_prepare(x, wq, wk, wv, wo, freqs_cos, freqs_sin, B, S):
    """Build per-core in_maps. Weights nn.Linear-style [out, in]."""
    NQB = S // 512
    NTG = B * NQB
    n_heads = wq.shape[0] // HD
    n_kv = wk.shape[0] // HD
    hpc = n_heads // N_CORES       # q heads per core (4)
    kpc = n_kv // N_CORES          # kv heads per core (1)

    # deinterleave rope pairs: feature order (2i) first then (2i+1), per head
    de = np.concatenate([np.arange(0, HD, 2), np.arange(1, HD, 2)])

    xf = np.ascontiguousarray(x.reshape(B * S, DIM))
    # x^T tiled: [g, kq, k, p, t]
    x_t = np.ascontiguousarray(
        xf.T.reshape(NKQ, KPQ, 128, NTG, 512).transpose(3, 0, 1, 2, 4)).astype(BF16NP)

    cos = np.repeat(freqs_cos, 2, axis=1)   # [S, 128] interleaved dup
    sin = np.repeat(freqs_sin, 2, axis=1)
    cc = cos[:, de]                                             # deinterleaved
    ss = sin.copy()
    ss[:, 0::2] *= -1.0                                         # [-sin, +sin]
    ss = ss[:, de]
    cct = np.ascontiguousarray(cc.T).astype(BF16NP)             # [128, S]
    # half-rotated: device row p holds sin-row (p+64)%128 (see rope multiplies)
    sst = np.ascontiguousarray(np.roll(ss.T, -64, axis=0)).astype(BF16NP)

    ident = np.eye(128, dtype=np.float32).astype(BF16NP)
    ones = np.ones((128, 128), dtype=np.float32).astype(BF16NP)
    # transposed-orientation causal mask for the diagonal 128-col wedge:
    # row r (sk within tile), col u (sq offset within wedge): keep iff u >= r
    r_idx = np.arange(128)[:, None]
    u_idx = np.arange(128)[None, :]
    maskt = np.where(u_idx >= r_idx, 0.0, -1e30).astype(np.float32)

    in_maps = []
    for cidx in range(N_CORES):
        qs = slice(cidx * hpc * HD, (cidx + 1) * hpc * HD)
        ks = slice(cidx * kpc * HD, (cidx + 1) * kpc * HD)
        wq_c = wq[qs].reshape(hpc, HD, DIM)[:, de, :].reshape(hpc * HD, DIM)
        wk_c = wk[ks].reshape(kpc, HD, DIM)[:, de, :].reshape(kpc * HD, DIM)
        wv_c = wv[ks]
        wkv_c = np.concatenate([wk_c, wv_c], axis=0)
        wo_c = wo[:, qs]
        in_maps.append({
            "x_t": x_t,
            "wq_t": np.ascontiguousarray(wq_c.T).astype(BF16NP),
            "wkv_t": np.ascontiguousarray(wkv_c.T).astype(BF16NP),
            "wo_t": np.ascontiguousarray(wo_c.T).astype(BF16NP),
            "cct": cct,
            "sst": sst,
            "ident": ident,
            "ones": ones,
            "maskt": maskt,
        })
    return in_maps


_CACHE = {}


def run(inputs, trace=False, trace_cores=None):
    x = np.asarray(inputs["x"], dtype=np.float32)
    B, S, _ = x.shape
    key = (B, S)
    if key not in _CACHE:
        _CACHE[key] = build_nc(B, S)
    nc = _CACHE[key]
    in_maps = host_prepare(
        x, np.asarray(inputs["wq"], np.float32), np.asarray(inputs["wk"], np.float32),
        np.asarray(inputs["wv"], np.float32), np.asarray(inputs["wo"], np.float32),
        np.asarray(inputs["freqs_cos"], np.float32),
        np.asarray(inputs["freqs_sin"], np.float32), B, S)
    res = bass_utils.run_bass_kernel_spmd(
        nc, in_maps, core_ids=list(range(N_CORES)), trace=trace,
        trace_cores=trace_cores)
    acc = np.zeros((B * S, DIM), dtype=np.float64)
    for r in res.results:
        acc += r["out"].astype(np.float64)
    out = acc.astype(np.float32).reshape(B, S, DIM)
    return out, res


def kernel(**inputs) -> np.ndarray:
    assert int(inputs.get("start_pos", 0)) == 0
    out, _ = run(inputs, trace=False)
    return out
